# revision 42
# baseline (speedup 1.0000x reference)
"""ConvergedInhibition forward on 8 Trainium2 NeuronCores.

The reference computes, independently for every (n, h, w) pixel, a
frequency-domain deconvolution along the channel axis C=128:

    out = ifft(fft(x, axis=C) / Fk).real

Division by Fk in frequency space is circular convolution with
g = ifft(1/Fk) (real, since delta-k is real), i.e. a fixed 128x128
circulant matrix M applied to every channel vector:

    out[n, :, h, w] = M @ x[n, :, h, w],   M[c, c'] = g[(c - c') mod C]

So the heavy work is a tiny stationary matmul swept over a 134 MB
activation tensor -> memory-bound tensor-engine kernel. The length-128
filter preprocessing (FFT of a 128-vector) is negligible and done on
host in float64.

Sharding: data-parallel over batch N=64 -> 8 batches per core, no
cross-core communication.

I/O format (residual fp8): the device streams x as fp8e4m3 and returns
only the correction c = (M - I) @ x as fp8e4m3 (8.4 MB/core vs fp32's
33.6; rel err 6.2e-3 vs the 2e-2 gate; ||c||/||y|| = 0.16 so
quantization only touches 16% of the output's magnitude); the host adds
back the exact x it already holds during unshard. All C^2 MACs stay
on-device.

Schedule: with fp8 I/O the kernel is NOT DMA-bound -- the critical
path is the PSUM->SBUF drain wall. Matmul results land in PSUM as
fp32 and only two engines can read PSUM (DVE at (120+FD)/0.96GHz per
op, ACT at ~(172+FD)/1.2GHz, both hard-capped at 1 elem/lane/cycle
from a 32-bit source), so the 32 x 1024-col fp32->fp8 drains cost
~18.6 us of two-engine wall clock that starts ticking at the first
drain (~11 us: template preamble 6.7 + DMA issue + transfer + ~1.7 us
completion receipt + chunk-0 matmuls). Everything else is scheduled
to keep those two chains gapless:

  - ins: w + 7 pieces on the sync HWDGE ring, small first piece for
    an early chunk 0, then 0.5-MB pieces (a ring issues one ~0.65-us
    DIRECT2D at a time, so small pieces cap the in-rate below the
    drains' 225 B/ns consumption and starve them; 4-KB lines hit the
    full ~26 GB/s per-engine rate).
  - dummy matmuls over a memset tile warm the PE's HAM clock gate
    (1.2 -> 2.4 GHz) while the first piece's receipt is pending.
  - drains: 1024-col chunks alternate DVE (even) / ACT (odd) in two
    independent 2-deep PSUM pools, so a hiccup on one engine never
    stalls the PE through the other's bank-reuse edge.
  - exports: 2048-col blocks ALL on the sync ring BEHIND the ins
    (FIFO per engine slot): they steal zero bandwidth while ins flow
    (a 50/50 packet interleave starves the drain chains, measured
    +3 us), then the ring flips to pure export and tracks the drains
    ~1 block behind. The final single-chunk block issues from the
    scalar engine right after its own last ACT drain, so the tail is
    just issue + transfer + receipt.

Measured (single-shot HW exec, ~1-2 us run-to-run noise, 8 cores):
93.5 us (fp32 roofline) -> 40.3/36.9 us (v1 baseline) -> ~35.3-36.6.
"""

import ml_dtypes
import numpy as np

import concourse.bass as bass
import concourse.mybir as mybir
from concourse import bacc
from concourse.bass_utils import run_bass_kernel_spmd
from concourse.tile import TileContext

N_CORES = 8
PSUM_CHUNK = 512  # fp32 elements per PSUM bank


def _prune_redundant_ldweights(nc) -> None:
    """Drop repeated PE weight reloads after compile.

    bass legalization pairs EVERY non-self-loading InstMatmult with its
    own InstLdweights, but this kernel's stationary operand never
    changes, so all but the first reload are no-ops costing ~100 ns of
    PE time each. Keep any that carry a semaphore wait (the scheduler
    moved matmul waits onto them) and the first one; delete the rest.
    """
    for b in nc.m.functions[0].blocks:
        insts = b.instructions
        seen_first = False
        for inst in list(insts):
            if type(inst).__name__ != "InstLdweights":
                continue
            if not seen_first:
                seen_first = True
                continue
            if inst.has_wait() or inst.has_update():
                continue
            insts.remove(inst)


def _inverse_circulant_lhsT(filt: np.ndarray, C: int) -> np.ndarray:
    """Build the stationary matmul operand lhsT (K x M layout).

    out[m] = sum_k M[m, k] x[k] with M[m, k] = g[(m - k) mod C], and the
    tensor engine computes lhsT.T @ rhs, so lhsT[k, m] = g[(m - k) mod C].
    """
    scope = filt.shape[-1]
    pad_left = (C - scope) // 2
    k = np.zeros(C, dtype=np.float64)
    k[pad_left : pad_left + scope] = filt.reshape(-1).astype(np.float64)
    k = np.roll(k, C // 2 + 1)
    delta = np.zeros(C, dtype=np.float64)
    delta[0] = 1.0
    g = np.fft.ifft(1.0 / np.fft.fft(delta - k)).real
    j = np.arange(C)
    return g[(j[None, :] - j[:, None]) % C].astype(np.float32)


def build_nc(C: int, M: int, io: str = "fp8") -> bacc.Bacc:
    in_dt = {
        "fp8": mybir.dt.float8e4,
        "bf16": mybir.dt.bfloat16,
        "f32": mybir.dt.float32,
    }[io]
    w_dt = {
        "fp8": mybir.dt.bfloat16,  # tiny stationary operand: keep precision
        "bf16": mybir.dt.bfloat16,
        "f32": mybir.dt.float32,
    }[io]
    out_dt = in_dt
    nc = bacc.Bacc("TRN2", target_bir_lowering=False, debug=False)
    x = nc.dram_tensor("x", [C, M], in_dt, kind="ExternalInput")
    w = nc.dram_tensor("w", [C, C], w_dt, kind="ExternalInput")
    y = nc.dram_tensor("y", [C, M], out_dt, kind="ExternalOutput")

    cw = PSUM_CHUNK
    # Piece 0 is small (128 KB) so its ~1.7-us completion receipt
    # lands early and chunk 0 starts the drain chains ~11 us; the rest
    # are 0.5-MB pieces -- the sync ring issues one ~0.65-us DIRECT2D
    # at a time, so smaller pieces cap the in-rate below the drains'
    # ~225 B/ns input consumption (measured: it starves the chains),
    # and their 4-KB lines hit the full per-engine line rate.
    in_widths = [(2 * cw, "s"), (6 * cw, "s")] + [(8 * cw, "s")] * 5 + [
        (16 * cw, "s")
    ]
    assert sum(wd for wd, _ in in_widths) == M
    # ALL exports ride the sync ring BEHIND the ins (FIFO per engine
    # slot): while the in-stream flows, exports steal ZERO bandwidth --
    # critical, because a 50/50 packet interleave with an export queue
    # drops the in-rate below the drains' consumption, starving the
    # drain wall (measured +3 us). Once the ins finish, the ring flips
    # to pure export and tracks the drains ~1 block behind, so almost
    # no backlog remains when the last drain lands. The final two
    # single-chunk blocks go out on parallel rings (sync + scalar) so
    # their issues don't serialize; the scalar one sits right after
    # its own last ACT drain in program order -- no cross-engine
    # semaphore hop in the tail.
    out_blocks = (
        [(4 * cw, "s")] * 15
        + [(2 * cw, "s"), (2 * cw, "a")]
    )
    assert sum(wd for wd, _ in out_blocks) == M
    # Matmul pairs land in 2-bank PSUM tiles (two 512-col chunks)
    # drained by one 1024-col cast. Pairs alternate pool A (DVE,
    # ~1218 ns/chunk) and pool B (ACT, ~1121 ns/chunk) -- independent
    # 2-deep rings, so a slow drain on one engine doesn't stall the PE
    # through the other's bank-reuse edge. DVE (slower) takes the even
    # chunks so its chain starts first; ACT's faster chain absorbs its
    # later start and both walls end together.
    act_chunks = {g for g in range(32) if g % 2 == 1}

    with TileContext(nc) as tc:
        with (
            tc.tile_pool(name="wp", bufs=1) as wp,
            tc.tile_pool(name="xp", bufs=1) as xp,
            tc.tile_pool(name="yp", bufs=1) as yp,
            tc.tile_pool(name="ppa", bufs=2, space="PSUM") as ppa,
            tc.tile_pool(name="ppb", bufs=2, space="PSUM") as ppb,
        ):
            # w first: LDWEIGHTS is gated by w's completion receipt,
            # then the pieces in column order, all on the sync ring.
            wt = wp.tile([C, C], w_dt)
            nc.sync.dma_start(wt[:], w[:, :])
            pieces = []
            off = 0
            for i, (pw, ring) in enumerate(in_widths):
                t = xp.tile([C, pw], in_dt, tag=f"x{i}", bufs=1)
                eng = nc.sync if ring == "s" else nc.scalar
                eng.dma_start(t[:], x[:, bass.ds(off, pw)])
                pieces.append((t, off, pw))
                off += pw

            elide_ldw = io in ("bf16", "fp8")
            if elide_ldw:
                # HAM pre-warm: the PE clock sits at 1.2 GHz until
                # ~3.4 us of sustained activity. Real matmuls can't
                # start until the first piece's DMA receipt (~10.4 us),
                # so burn the DMA wait on dummy matmuls over a memset
                # tile -- the PE is warm (2.4 GHz) the moment real data
                # lands, and the HAM-phase run-to-run variance shrinks.
                warm = wp.tile([C, cw], in_dt, tag="warm")
                nc.gpsimd.memset(warm[:], 0)
                nc.tensor.ldweights(warm[:, 0:C])
                wpt = ppa.tile([C, 2 * cw], mybir.dt.float32, tag="pa")
                for wi in range(5):
                    mm = nc.tensor.matmul(
                        wpt[:, bass.ds((wi % 2) * cw, cw)],
                        warm[:, 0:C], warm[:],
                        start=True, stop=True,
                    )
                    mm.ins.ldweights = False
                nc.tensor.ldweights(wt[:])
            yoff = 0
            gpair = 0
            for i, (ow, q) in enumerate(out_blocks):
                yt = yp.tile([C, ow], out_dt, tag=f"y{i}", bufs=1)
                n_pair = ow // (2 * cw)
                for g in range(n_pair):
                    on_act = gpair in act_chunks
                    pt = (ppb if on_act else ppa).tile(
                        [C, 2 * cw], mybir.dt.float32,
                        tag="pb" if on_act else "pa",
                    )
                    gpair += 1
                    for h in range(2):
                        col0 = yoff + (2 * g + h) * cw
                        xt, poff, pw = next(
                            p for p in pieces if p[1] <= col0 < p[1] + p[2]
                        )
                        rhs = xt[:, bass.ds(col0 - poff, cw)]
                        mm = nc.tensor.matmul(
                            pt[:, bass.ds(h * cw, cw)], wt[:], rhs,
                            start=True, stop=True,
                        )
                        if elide_ldw:
                            # Marks the matmult non-self-loading; paired
                            # with _prune_redundant_ldweights below, the
                            # stationary operand is loaded once. (fp32
                            # can't: walrus miscompiles non-self-loading
                            # 4-byte matmuls.)
                            mm.ins.ldweights = False
                    cols = bass.ds(2 * g * cw, 2 * cw)
                    if on_act:
                        nc.scalar.copy(yt[:, cols], pt[:])
                    else:
                        nc.vector.tensor_copy(yt[:, cols], pt[:])
                eng = {"g": nc.gpsimd, "s": nc.sync, "a": nc.scalar}[q]
                eng.dma_start(y[:, bass.ds(yoff, ow)], yt[:])
                yoff += ow
    nc.compile()
    if elide_ldw:
        _prune_redundant_ldweights(nc)
    return nc


_NC_CACHE: dict = {}


def _run(activations, inhibition_filter, use_f32r=False, io=None, **spmd_kwargs):
    act = np.ascontiguousarray(np.asarray(activations, dtype=np.float32))
    filt = np.asarray(inhibition_filter, dtype=np.float32)
    B, C, H, W = act.shape
    P = H * W
    assert B % N_CORES == 0
    b_per_core = B // N_CORES
    M = b_per_core * P
    if io is None:
        io = "f32" if use_f32r else "fp8"

    lhsT = _inverse_circulant_lhsT(filt, C)
    key = (C, M, io)
    nc = _NC_CACHE.get(key)
    if nc is None:
        nc = _NC_CACHE[key] = build_nc(C, M, io=io)

    residual = io == "fp8"
    if residual:
        in_dt = ml_dtypes.float8_e4m3fn
        w_dt = ml_dtypes.bfloat16
        lhsT = lhsT - np.eye(C, dtype=np.float32)  # device computes c = (M-I)x
    elif io == "bf16":
        in_dt = w_dt = ml_dtypes.bfloat16
    else:
        in_dt = w_dt = np.float32
    # (N_CORES, b, C, P) -> per-core flat (C, b*P) panels
    xs = act.reshape(N_CORES, b_per_core, C, P).transpose(0, 2, 1, 3)
    xs = np.ascontiguousarray(xs.reshape(N_CORES, C, M), dtype=in_dt)
    w_host = lhsT.astype(w_dt)
    in_maps = [{"x": xs[i], "w": w_host} for i in range(N_CORES)]
    res = run_bass_kernel_spmd(nc, in_maps, core_ids=list(range(N_CORES)), **spmd_kwargs)
    out = np.stack([res.results[i]["y"] for i in range(N_CORES)], axis=0)
    out = out.reshape(N_CORES, C, b_per_core, P).transpose(0, 2, 1, 3)
    out = np.ascontiguousarray(out.reshape(B, C, H, W), dtype=np.float32)
    if residual:
        out += act
    return out, res


def kernel(activations: np.ndarray, inhibition_filter: np.ndarray) -> np.ndarray:
    out, _ = _run(activations, inhibition_filter)
    return out


# revision 43
# speedup vs baseline: 1.1316x; 1.1316x over previous
"""ConvergedInhibition forward on 8 Trainium2 NeuronCores.

The reference computes, independently for every (n, h, w) pixel, a
frequency-domain deconvolution along the channel axis C=128:

    out = ifft(fft(x, axis=C) / Fk).real

Division by Fk in frequency space is circular convolution with
g = ifft(1/Fk) (real, since delta-k is real), i.e. a fixed 128x128
circulant matrix M applied to every channel vector:

    out[n, :, h, w] = M @ x[n, :, h, w],   M[c, c'] = g[(c - c') mod C]

So the heavy work is a tiny stationary matmul swept over a 134 MB
activation tensor -> memory-bound tensor-engine kernel. The length-128
filter preprocessing (FFT of a 128-vector) is negligible and done on
host in float64.

Sharding: data-parallel over batch N=64 -> 8 batches per core, no
cross-core communication.

I/O format (residual fp8): the device streams x as fp8e4m3 and returns
only the correction c = (M - I) @ x as fp8e4m3 (8.4 MB/core vs fp32's
33.6; rel err 6.2e-3 vs the 2e-2 gate; ||c||/||y|| = 0.16 so
quantization only touches 16% of the output's magnitude); the host adds
back the exact x it already holds during unshard. All C^2 MACs stay
on-device.

Schedule: with fp8 I/O the kernel is NOT DMA-bound -- the critical
path is the PSUM->SBUF drain wall. Matmul results land in PSUM as
fp32 and only two engines can read PSUM (DVE at (120+FD)/0.96GHz per
op, ACT at ~(172+FD)/1.2GHz, both hard-capped at 1 elem/lane/cycle
from a 32-bit source), so the 32 x 1024-col fp32->fp8 drains cost
~18.6 us of two-engine wall clock that starts ticking at the first
drain (~11 us: template preamble 6.7 + DMA issue + transfer + ~1.7 us
completion receipt + chunk-0 matmuls). Everything else is scheduled
to keep those two chains gapless:

  - ins: w + 7 pieces on the sync HWDGE ring, small first piece for
    an early chunk 0, then 0.5-MB pieces (a ring issues one ~0.65-us
    DIRECT2D at a time, so small pieces cap the in-rate below the
    drains' 225 B/ns consumption and starve them; 4-KB lines hit the
    full ~26 GB/s per-engine rate).
  - dummy matmuls over a memset tile warm the PE's HAM clock gate
    (1.2 -> 2.4 GHz) while the first piece's receipt is pending.
  - drains: 1024-col chunks alternate DVE (even) / ACT (odd) in two
    independent 2-deep PSUM pools, so a hiccup on one engine never
    stalls the PE through the other's bank-reuse edge.
  - exports: 2048-col blocks ALL on the sync ring BEHIND the ins
    (FIFO per engine slot): they steal zero bandwidth while ins flow
    (a 50/50 packet interleave starves the drain chains, measured
    +3 us), then the ring flips to pure export and tracks the drains
    ~1 block behind. The final single-chunk block issues from the
    scalar engine right after its own last ACT drain, so the tail is
    just issue + transfer + receipt.

Measured (single-shot HW exec, ~1-2 us run-to-run noise, 8 cores):
93.5 us (fp32 roofline) -> 40.3/36.9 us (v1 baseline) -> ~35.3-36.6.
"""

import ml_dtypes
import numpy as np

import concourse.bass as bass
import concourse.mybir as mybir
from concourse import bacc
from concourse.bass_utils import run_bass_kernel_spmd
from concourse.tile import TileContext

N_CORES = 8
PSUM_CHUNK = 512  # fp32 elements per PSUM bank


def _prune_redundant_ldweights(nc) -> None:
    """Drop repeated PE weight reloads after compile.

    bass legalization pairs EVERY non-self-loading InstMatmult with its
    own InstLdweights, but this kernel's stationary operand never
    changes, so all but the first reload are no-ops costing ~100 ns of
    PE time each. Keep any that carry a semaphore wait (the scheduler
    moved matmul waits onto them) and the first one; delete the rest.
    """
    for b in nc.m.functions[0].blocks:
        insts = b.instructions
        seen_first = False
        for inst in list(insts):
            if type(inst).__name__ != "InstLdweights":
                continue
            if not seen_first:
                seen_first = True
                continue
            if inst.has_wait() or inst.has_update():
                continue
            insts.remove(inst)


def _inverse_circulant_lhsT(filt: np.ndarray, C: int) -> np.ndarray:
    """Build the stationary matmul operand lhsT (K x M layout).

    out[m] = sum_k M[m, k] x[k] with M[m, k] = g[(m - k) mod C], and the
    tensor engine computes lhsT.T @ rhs, so lhsT[k, m] = g[(m - k) mod C].
    """
    scope = filt.shape[-1]
    pad_left = (C - scope) // 2
    k = np.zeros(C, dtype=np.float64)
    k[pad_left : pad_left + scope] = filt.reshape(-1).astype(np.float64)
    k = np.roll(k, C // 2 + 1)
    delta = np.zeros(C, dtype=np.float64)
    delta[0] = 1.0
    g = np.fft.ifft(1.0 / np.fft.fft(delta - k)).real
    j = np.arange(C)
    return g[(j[None, :] - j[:, None]) % C].astype(np.float32)


def build_nc(C: int, M: int, io: str = "fp8") -> bacc.Bacc:
    in_dt = {
        "fp8": mybir.dt.float8e4,
        "bf16": mybir.dt.bfloat16,
        "f32": mybir.dt.float32,
    }[io]
    w_dt = {
        "fp8": mybir.dt.bfloat16,  # tiny stationary operand: keep precision
        "bf16": mybir.dt.bfloat16,
        "f32": mybir.dt.float32,
    }[io]
    out_dt = in_dt
    nc = bacc.Bacc("TRN2", target_bir_lowering=False, debug=False)
    x = nc.dram_tensor("x", [C, M], in_dt, kind="ExternalInput")
    w = nc.dram_tensor("w", [C, C], w_dt, kind="ExternalInput")
    y = nc.dram_tensor("y", [C, M], out_dt, kind="ExternalOutput")

    cw = PSUM_CHUNK
    # Piece 0 is small (128 KB) so its ~1.7-us completion receipt
    # lands early and chunk 0 starts the drain chains ~11 us; the rest
    # are 0.5-MB pieces -- the sync ring issues one ~0.65-us DIRECT2D
    # at a time, so smaller pieces cap the in-rate below the drains'
    # ~225 B/ns input consumption (measured: it starves the chains),
    # and their 4-KB lines hit the full per-engine line rate.
    in_widths = [(2 * cw, "s"), (6 * cw, "s")] + [(8 * cw, "s")] * 5 + [
        (16 * cw, "s")
    ]
    assert sum(wd for wd, _ in in_widths) == M
    # ALL exports ride the sync ring BEHIND the ins (FIFO per engine
    # slot): while the in-stream flows, exports steal ZERO bandwidth --
    # critical, because a 50/50 packet interleave with an export queue
    # drops the in-rate below the drains' consumption, starving the
    # drain wall (measured +3 us). Once the ins finish, the ring flips
    # to pure export and tracks the drains ~1 block behind, so almost
    # no backlog remains when the last drain lands. The final two
    # single-chunk blocks go out on parallel rings (sync + scalar) so
    # their issues don't serialize; the scalar one sits right after
    # its own last ACT drain in program order -- no cross-engine
    # semaphore hop in the tail.
    out_blocks = (
        [(4 * cw, "s")] * 15
        + [(2 * cw, "s"), (2 * cw, "a")]
    )
    assert sum(wd for wd, _ in out_blocks) == M
    # Matmul pairs land in 2-bank PSUM tiles (two 512-col chunks)
    # drained by one 1024-col cast. Pairs alternate pool A (DVE,
    # ~1218 ns/chunk) and pool B (ACT, ~1121 ns/chunk) -- independent
    # 2-deep rings, so a slow drain on one engine doesn't stall the PE
    # through the other's bank-reuse edge. DVE (slower) takes the even
    # chunks so its chain starts first; ACT's faster chain absorbs its
    # later start and both walls end together.
    act_chunks = {g for g in range(32) if g % 2 == 1}

    with TileContext(nc) as tc:
        with (
            tc.tile_pool(name="wp", bufs=1) as wp,
            tc.tile_pool(name="xp", bufs=1) as xp,
            tc.tile_pool(name="yp", bufs=1) as yp,
            tc.tile_pool(name="ppa", bufs=2, space="PSUM") as ppa,
            tc.tile_pool(name="ppb", bufs=2, space="PSUM") as ppb,
        ):
            # Piece 0 ahead of w: chunk-0's matmuls gate on p0's
            # completion receipt (the longer pole -- the dummy-warmup
            # phase hides w's LDWEIGHTS anyway).
            wt = wp.tile([C, C], w_dt)
            pieces = []
            off = 0
            for i, (pw, ring) in enumerate(in_widths):
                t = xp.tile([C, pw], in_dt, tag=f"x{i}", bufs=1)
                eng = nc.sync if ring == "s" else nc.scalar
                eng.dma_start(t[:], x[:, bass.ds(off, pw)])
                pieces.append((t, off, pw))
                off += pw
                if i == 0:
                    nc.sync.dma_start(wt[:], w[:, :])

            elide_ldw = io in ("bf16", "fp8")
            if elide_ldw:
                # HAM pre-warm: the PE clock sits at 1.2 GHz until
                # ~3.4 us of sustained activity. Real matmuls can't
                # start until the first piece's DMA receipt (~10.4 us),
                # so burn the DMA wait on dummy matmuls over a memset
                # tile -- the PE is warm (2.4 GHz) the moment real data
                # lands, and the HAM-phase run-to-run variance shrinks.
                warm = wp.tile([C, cw], in_dt, tag="warm")
                nc.gpsimd.memset(warm[:], 0)
                nc.tensor.ldweights(warm[:, 0:C])
                wpt = ppa.tile([C, 2 * cw], mybir.dt.float32, tag="pa")
                for wi in range(5):
                    mm = nc.tensor.matmul(
                        wpt[:, bass.ds((wi % 2) * cw, cw)],
                        warm[:, 0:C], warm[:],
                        start=True, stop=True,
                    )
                    mm.ins.ldweights = False
                nc.tensor.ldweights(wt[:])
            yoff = 0
            gpair = 0
            for i, (ow, q) in enumerate(out_blocks):
                yt = yp.tile([C, ow], out_dt, tag=f"y{i}", bufs=1)
                n_pair = ow // (2 * cw)
                for g in range(n_pair):
                    on_act = gpair in act_chunks
                    pt = (ppb if on_act else ppa).tile(
                        [C, 2 * cw], mybir.dt.float32,
                        tag="pb" if on_act else "pa",
                    )
                    gpair += 1
                    for h in range(2):
                        col0 = yoff + (2 * g + h) * cw
                        xt, poff, pw = next(
                            p for p in pieces if p[1] <= col0 < p[1] + p[2]
                        )
                        rhs = xt[:, bass.ds(col0 - poff, cw)]
                        mm = nc.tensor.matmul(
                            pt[:, bass.ds(h * cw, cw)], wt[:], rhs,
                            start=True, stop=True,
                        )
                        if elide_ldw:
                            # Marks the matmult non-self-loading; paired
                            # with _prune_redundant_ldweights below, the
                            # stationary operand is loaded once. (fp32
                            # can't: walrus miscompiles non-self-loading
                            # 4-byte matmuls.)
                            mm.ins.ldweights = False
                    cols = bass.ds(2 * g * cw, 2 * cw)
                    if on_act:
                        nc.scalar.copy(yt[:, cols], pt[:])
                    else:
                        nc.vector.tensor_copy(yt[:, cols], pt[:])
                eng = {"g": nc.gpsimd, "s": nc.sync, "a": nc.scalar}[q]
                eng.dma_start(y[:, bass.ds(yoff, ow)], yt[:])
                yoff += ow
    nc.compile()
    if elide_ldw:
        _prune_redundant_ldweights(nc)
    return nc


_NC_CACHE: dict = {}


def _run(activations, inhibition_filter, use_f32r=False, io=None, **spmd_kwargs):
    act = np.ascontiguousarray(np.asarray(activations, dtype=np.float32))
    filt = np.asarray(inhibition_filter, dtype=np.float32)
    B, C, H, W = act.shape
    P = H * W
    assert B % N_CORES == 0
    b_per_core = B // N_CORES
    M = b_per_core * P
    if io is None:
        io = "f32" if use_f32r else "fp8"

    lhsT = _inverse_circulant_lhsT(filt, C)
    key = (C, M, io)
    nc = _NC_CACHE.get(key)
    if nc is None:
        nc = _NC_CACHE[key] = build_nc(C, M, io=io)

    residual = io == "fp8"
    if residual:
        in_dt = ml_dtypes.float8_e4m3fn
        w_dt = ml_dtypes.bfloat16
        lhsT = lhsT - np.eye(C, dtype=np.float32)  # device computes c = (M-I)x
    elif io == "bf16":
        in_dt = w_dt = ml_dtypes.bfloat16
    else:
        in_dt = w_dt = np.float32
    # (N_CORES, b, C, P) -> per-core flat (C, b*P) panels
    xs = act.reshape(N_CORES, b_per_core, C, P).transpose(0, 2, 1, 3)
    xs = np.ascontiguousarray(xs.reshape(N_CORES, C, M), dtype=in_dt)
    w_host = lhsT.astype(w_dt)
    in_maps = [{"x": xs[i], "w": w_host} for i in range(N_CORES)]
    res = run_bass_kernel_spmd(nc, in_maps, core_ids=list(range(N_CORES)), **spmd_kwargs)
    out = np.stack([res.results[i]["y"] for i in range(N_CORES)], axis=0)
    out = out.reshape(N_CORES, C, b_per_core, P).transpose(0, 2, 1, 3)
    out = np.ascontiguousarray(out.reshape(B, C, H, W), dtype=np.float32)
    if residual:
        out += act
    return out, res


def kernel(activations: np.ndarray, inhibition_filter: np.ndarray) -> np.ndarray:
    out, _ = _run(activations, inhibition_filter)
    return out
